# revision 1
# baseline (speedup 1.0000x reference)
"""BandLinear kernel for 8 TRN2 NeuronCores — shifted-window scheme.

out[n, o] = sum_i x[n, i] * (weight * mask)[o, i] + bias[o]
with a +-8 band mask, x: [16384, 4096] f32.

Strategy (data-parallel over tokens, 2048 tokens/core):
 - Host packs each core's x shard transposed AND shifted by -8 features:
     xt[p, 2048*j + n] = x[n, 128*j - 8 + p]   (0 when feature < 0)
   so output block j (outputs [128j, 128j+128)) gets its inputs
   [128j-8, 128j+120) from ONE K=128 stationary matmul; the missing
   right-halo inputs [128j+120, 128j+136) are partitions 0..15 of
   block j+1. A tiny xtl[16, 2048] tensor carries the features >= 4088
   that the shift drops from block 31's halo.
 - Weights shrink to the truly-needed couplings: wm [128, 32*128]
   (main) + wh [16, 32*128] (halo) ~ 1.15 MB bf16 vs 3.1 MB for the
   block-tridiagonal packing (most of those blocks are zeros).
 - The halo stationaries are zero-padded on-chip to K=128 (memset +
   DMA into partitions 0..15) and the halo moving slices read all 128
   partitions of block j+1: keeping every matmul at tile_size
   (128, 128) lets the PE overlap LDWEIGHTS with matmuls and stay in
   the HAM-warm 2.4 GHz state (v2's alternating (32,128)/(128,128)
   configs serialized LS and pinned the PE cold at 1.2 GHz ->
   157 us). Same-stationary matmuls are grouped (4 token chunks per
   LDWEIGHTS) to maximize reuse.
 - PE work: 2 matmuls per (j, token-chunk) instead of 3 (131K vs 196K
   columns -> ~55 us vs ~82 us @2.4 GHz), giving scheduling slack under
   the ~97 us DMA floor (34.9 MB/core @ ~358 GB/s).
 - DMA queues: x loads on nc.sync (SP HWDGE ring); out stores on
   nc.gpsimd (SWDGE) so a store waiting on compute never head-of-line
   blocks an x prefetch.
 - Bias fused into the PSUM->SBUF drain (alternating ScalarE/VectorE).
 - Device writes out^T in j-pair tiles [128, 2*2048] bf16; host
   un-transposes and upcasts.
"""

import os
import sys

for _p in ("/opt/trn_rl_repo", "/root/.axon_site/_ro/trn_rl_repo"):
    if os.path.isdir(_p) and _p not in sys.path:
        sys.path.append(_p)

import numpy as np
import ml_dtypes

import concourse.bacc as bacc
import concourse.mybir as mybir
from concourse.bass_utils import run_bass_kernel_spmd
from concourse.tile import TileContext

N_CORES = 8
N_TOK = 16384
NF = 4096
BAND = 8
TPC = N_TOK // N_CORES          # tokens per core (2048)
KB = NF // 128                  # 32 feature blocks
CC = TPC // 512                 # token chunks of 512 per core (4)

COMPUTE_DT = os.environ.get("BAND_COMPUTE_DT", "bfloat16")
OUT_DT = os.environ.get("BAND_OUT_DT", "bfloat16")

LAST_RESULT = None  # BassKernelResults of the most recent run (for test.py)

_cache = {}


def _np_dt(name):
    return ml_dtypes.bfloat16 if name == "bfloat16" else np.float32


def _build(compute_dt: str, out_dt: str):
    cdt = getattr(mybir.dt, compute_dt)
    odt = getattr(mybir.dt, out_dt)
    f32 = mybir.dt.float32
    nc = bacc.Bacc("TRN2", target_bir_lowering=False, debug=False,
                   num_devices=N_CORES)
    NQ = KB // 2                   # 16 j-pair chunks
    # xt[q, p, 2048*jj + n] = x[n, 128*(2q+jj) - 8 + p]
    XT = nc.dram_tensor("xt", [NQ, 128, 2 * TPC], cdt,
                        kind="ExternalInput").ap()
    # xtl[p, n] = x[n, 4088 + p] for p < 8 else 0 (zero-padded to 128
    # partitions on-chip; loaded late, when the read queue is draining)
    XTL = nc.dram_tensor("xtl", [16, TPC], cdt, kind="ExternalInput").ap()
    # wm[p, 128*j + m] = (weight*mask)[128j + m, 128j - 8 + p]
    WM = nc.dram_tensor("wm", [128, KB * 128], cdt,
                        kind="ExternalInput").ap()
    # wh[p, 128*j + m] = (weight*mask)[128j + m, 128j + 120 + p]
    # (zero-padded to K=128 on-chip: DVE memset + DMA into rows 0..15)
    WH = nc.dram_tensor("wh", [16, KB * 128], cdt,
                        kind="ExternalInput").ap()
    BM = nc.dram_tensor("bm", [128, KB], f32, kind="ExternalInput").ap()
    # ou[u, p, jj*TPC + n] = out^T[128*(2u+jj) + p, n]
    OU = nc.dram_tensor("out", [NQ, 128, 2 * TPC], odt,
                        kind="ExternalOutput").ap()

    ident = mybir.ActivationFunctionType.Identity
    add = mybir.AluOpType.add
    WMC = KB * 128 // 4            # wm column chunk (8 j's worth)

    with TileContext(nc) as tc:
        with (
            tc.tile_pool(name="bp", bufs=1) as bp,
            tc.tile_pool(name="xp", bufs=12) as xp,
            tc.tile_pool(name="op", bufs=7) as op,
            tc.tile_pool(name="pp", bufs=8, space="PSUM") as pp,
        ):
            bias_sb = bp.tile([128, KB], f32)
            wm_sb = bp.tile([128, KB * 128], cdt)
            wh_sb = bp.tile([128, KB * 128], cdt)
            xtl_sb = bp.tile([128, TPC], cdt)

            warm_sb = bp.tile([128, 512], cdt)
            # Zero-pad the halo operands to full 128 partitions (keeps
            # every matmul at tile_size (128, 128)). First DVE
            # instructions; they finish (~5 us) before the wh/xtl DMAs
            # can even dispatch, so nothing ever waits on them.
            nc.vector.memset(warm_sb[:], 0)
            nc.vector.memset(wh_sb[:], 0)
            nc.vector.memset(xtl_sb[:], 0)

            xq_sb = {}

            def load_xq(q):
                t = xp.tile([128, 2 * TPC], cdt, tag="xq")
                if q == 0:
                    # split so block j=0's matmuls start as soon as the
                    # first half-MB lands
                    nc.sync.dma_start(out=t[:, 0:TPC], in_=XT[0][:, 0:TPC])
                    nc.sync.dma_start(out=t[:, TPC:2 * TPC],
                                      in_=XT[0][:, TPC:2 * TPC])
                else:
                    nc.sync.dma_start(out=t[:], in_=XT[q])
                xq_sb[q] = t

            def x_slice(j, c):
                t = xq_sb[j // 2]
                base = (j % 2) * TPC + 512 * c
                return t[:, base:base + 512]

            def halo_slice(j, c):
                # inputs [128j+120, 128j+136) = partitions 0..15 of
                # block j+1 (xtl for j = KB-1); rows 16..127 are read
                # too but multiplied by the zero-padded weight rows.
                if j == KB - 1:
                    return xtl_sb[:, 512 * c:512 * c + 512]
                t = xq_sb[(j + 1) // 2]
                base = ((j + 1) % 2) * TPC + 512 * c
                return t[:, base:base + 512]

            # 12 reads dispatch upfront with no compute dependencies (x0
            # first — it gates the first matmul); the last 4 x loads and
            # the out stores interleave in the loop so store dispatches
            # reach the DMA ring ~20 us earlier than a fully front-
            # loaded read program would allow.
            load_xq(0)
            nc.sync.dma_start(out=wm_sb[:, 0:WMC], in_=WM[:, 0:WMC])
            nc.sync.dma_start(out=bias_sb[:], in_=BM[:])
            load_xq(1)
            load_xq(2)
            nc.sync.dma_start(out=wh_sb[0:16, :], in_=WH[:])
            load_xq(3)
            load_xq(4)
            nc.sync.dma_start(out=wm_sb[:, WMC:2 * WMC], in_=WM[:, WMC:2 * WMC])
            load_xq(5)
            nc.sync.dma_start(out=wm_sb[:, 2 * WMC:3 * WMC],
                              in_=WM[:, 2 * WMC:3 * WMC])
            load_xq(6)
            nc.sync.dma_start(out=wm_sb[:, 3 * WMC:4 * WMC],
                              in_=WM[:, 3 * WMC:4 * WMC])
            for q in range(7, 12):
                load_xq(q)

            # PE warm-up: dummy matmuls on the zeroed scratch tile fill
            # the otherwise-idle 1..10 us window so the HAM clock gate
            # reaches 2.4 GHz before the first real matmul; a few more
            # are scattered into the early read-fed pairs, whose feed
            # gaps otherwise re-trip the cold state (seen 20-27 us).
            def warm(n, tag):
                for w in range(n):
                    pw = pp.tile([128, 512], f32, tag="ps",
                                 name=f"warm{tag}_{w}")
                    nc.tensor.matmul(pw[:], warm_sb[:, 0:128], warm_sb[:],
                                     start=True, stop=True)

            warm(22, "a")

            oj2 = None
            for j in range(KB):
                q = j // 2
                if j in (4, 8):
                    warm(2, f"b{j}")
                if j % 2 == 0 and q + 12 < NQ:
                    load_xq(q + 12)
                if j == 18:
                    # read queue is draining; xtl well before j=31
                    nc.sync.dma_start(out=xtl_sb[0:16, :], in_=XTL[:])
                if j % 2 == 0:
                    oj2 = op.tile([128, 2 * TPC], odt, tag="o")
                wmj = wm_sb[:, 128 * j:128 * j + 128]
                whj = wh_sb[:, 128 * j:128 * j + 128]
                ps = [pp.tile([128, 512], f32, tag="ps", name=f"ps{j}_{c}")
                      for c in range(CC)]
                for c in range(CC):
                    nc.tensor.matmul(ps[c][:], wmj, x_slice(j, c),
                                     start=True, stop=False)
                for c in range(CC):
                    nc.tensor.matmul(ps[c][:], whj, halo_slice(j, c),
                                     start=False, stop=True)
                for c in range(CC):
                    ob = (j % 2) * TPC + 512 * c
                    osl = oj2[:, ob:ob + 512]
                    bsl = bias_sb[:, j:j + 1]
                    if (j + c) % 2 == 0:
                        nc.scalar.activation(osl, ps[c][:], ident, bias=bsl)
                    else:
                        nc.vector.tensor_scalar(osl, ps[c][:], bsl, None,
                                                op0=add)
                if j >= KB - 6:
                    # per-j half stores in the compute-paced tail keep
                    # the (otherwise read-empty) DMA queue supplied
                    jj = j % 2
                    nc.sync.dma_start(out=OU[q][:, jj * TPC:(jj + 1) * TPC],
                                      in_=oj2[:, jj * TPC:(jj + 1) * TPC])
                elif j % 2 == 1:
                    # HWDGE (sync) store: SWDGE (gpsimd) stores poison
                    # SDMA engines 7/15 via descriptor-ring AXI-port
                    # contention, delaying every read's completion
                    # semaphore by 12-16 us (measured).
                    nc.sync.dma_start(out=OU[q], in_=oj2[:])
    nc.finalize()
    return nc


def _get_nc(compute_dt, out_dt):
    key = (compute_dt, out_dt)
    if key not in _cache:
        _cache[key] = _build(compute_dt, out_dt)
    return _cache[key]


def kernel(x, weight, bias, mask):
    global LAST_RESULT
    x = np.asarray(x, dtype=np.float32)
    weight = np.asarray(weight, dtype=np.float32)
    bias = np.asarray(bias, dtype=np.float32)
    mask = np.asarray(mask, dtype=np.float32)

    cnp = _np_dt(COMPUTE_DT)
    wm_full = weight * mask                 # [O, I]

    # wm[p, 128j + m] = wm_full[128j + m, 128j - 8 + p]
    wm = np.zeros((128, KB, 128), dtype=np.float32)
    # wh[p, 128j + m] = wm_full[128j + m, 128j + 120 + p]
    wh = np.zeros((16, KB, 128), dtype=np.float32)
    for j in range(KB):
        blk = wm_full[128 * j:128 * j + 128]            # [128, NF]
        lo = 128 * j - 8
        s = max(0, -lo)
        wm[s:, j, :] = blk[:, lo + s:lo + 128].T
        hi = 128 * j + 120
        e = min(16, NF - hi)
        if e > 0:
            wh[:e, j, :] = blk[:, hi:hi + e].T
    wm = np.ascontiguousarray(wm.reshape(128, KB * 128).astype(cnp))
    wh = np.ascontiguousarray(wh.reshape(16, KB * 128).astype(cnp))

    bm = np.ascontiguousarray(bias.reshape(KB, 128).T.astype(np.float32))

    in_maps = []
    for ci in range(N_CORES):
        xs = x[TPC * ci:TPC * (ci + 1)]               # [TPC, NF]
        xsh = np.zeros((NF + 8, TPC), dtype=np.float32)
        xsh[8:] = xs.T                                 # xsh[8 + f, n] = xs[n, f]
        # xt[q, p, 2048*jj + n] = xsh[128*(2q+jj) + p, n]
        xt = (xsh[:NF].reshape(KB // 2, 2, 128, TPC)
              .transpose(0, 2, 1, 3))
        xt = np.ascontiguousarray(xt.astype(cnp)).reshape(KB // 2, 128,
                                                          2 * TPC)
        xtl = np.zeros((16, TPC), dtype=np.float32)
        xtl[:8] = xs.T[4088:]
        in_maps.append({
            "xt": xt,
            "xtl": np.ascontiguousarray(xtl.astype(cnp)),
            "wm": wm, "wh": wh, "bm": bm,
        })

    nc = _get_nc(COMPUTE_DT, OUT_DT)
    LAST_RESULT = run_bass_kernel_spmd(nc, in_maps, list(range(N_CORES)))

    out = np.empty((N_TOK, NF), dtype=np.float32)
    for ci in range(N_CORES):
        ou = np.asarray(LAST_RESULT.results[ci]["out"], dtype=np.float32)
        ot = (ou.reshape(KB // 2, 128, 2, TPC).transpose(0, 2, 1, 3)
              .reshape(NF, TPC))
        out[TPC * ci:TPC * (ci + 1)] = ot.T
    return out



# revision 2
# speedup vs baseline: 1.6121x; 1.6121x over previous
"""BandLinear kernel for 8 TRN2 NeuronCores — shifted-window scheme,
fp8(e3m4) x-side traffic.

out[n, o] = sum_i x[n, i] * (weight * mask)[o, i] + bias[o]
with a +-8 band mask, x: [16384, 4096] f32.

Strategy (data-parallel over tokens, 2048 tokens/core):
 - Host packs each core's x shard transposed AND shifted by -8 features:
     xt[p, 2048*j + n] = x[n, 128*j - 8 + p]   (0 when feature < 0)
   so output block j (outputs [128j, 128j+128)) gets its inputs
   [128j-8, 128j+120) from ONE K=128 stationary matmul; the missing
   right-halo inputs [128j+120, 128j+136) are partitions 0..15 of
   block j+1. A tiny xtl[16, 2048] tensor carries the features >= 4088
   that the shift drops from block 31's halo.
 - x ships as float8 e3m4 (1 byte): the TRN2 PE consumes e3m4 moving
   operands against a bf16 stationary at full rate with exact gradual
   underflow (probed: HW == dequantized-f32 emulation to 4.5e-8), and
   the measured end-to-end max-rel error vs the f32 reference is
   1.53e-2 (x-quantization dominated), under the 2e-2 gate. This
   halves the dominant read stream: 34.9 -> 25.7 MB/core, a ~72 us
   DMA floor at ~358 GB/s/core.
 - Weights stay bf16 and shrink to the truly-needed couplings:
   wm [128, 32*128] (main) + wh [16, 32*128] (halo). The halo
   stationaries are zero-padded on-chip to K=128 (memset + DMA into
   partitions 0..15): keeping every matmul at tile_size (128, 128)
   lets the PE overlap LDWEIGHTS with matmuls and stay in the HAM-warm
   2.4 GHz state. Same-stationary matmuls are grouped (4 token chunks
   per LDWEIGHTS) to maximize reuse.
 - PE work: 2 matmuls per (j, token-chunk), 131K columns ~ 55 us
   @2.4 GHz, under the ~72 us DMA floor.
 - DMA: all 16 x reads (512 KB each) front-load on the nc.sync HWDGE
   ring (xq tiles stay SBUF-resident, 64 KB/partition), so out stores
   paced by compute can never head-of-line block a read. Stores also
   go on nc.sync: SWDGE (gpsimd) stores poison SDMA engines 7/15 via
   descriptor-ring AXI-port contention (measured 12-16 us).
 - Bias fused into the PSUM->SBUF drain (alternating ScalarE/VectorE).
 - Device writes out^T in j-pair tiles [128, 2*2048] bf16; host
   un-transposes and upcasts.
"""

import os
import sys

for _p in ("/opt/trn_rl_repo", "/root/.axon_site/_ro/trn_rl_repo"):
    if os.path.isdir(_p) and _p not in sys.path:
        sys.path.append(_p)

import numpy as np
import ml_dtypes

import concourse.bacc as bacc
import concourse.mybir as mybir
from concourse.bass_utils import run_bass_kernel_spmd
from concourse.tile import TileContext

N_CORES = 8
N_TOK = 16384
NF = 4096
BAND = 8
TPC = N_TOK // N_CORES          # tokens per core (2048)
KB = NF // 128                  # 32 feature blocks
CC = TPC // 512                 # token chunks of 512 per core (4)

COMPUTE_DT = "bfloat16"         # weight dtype
X_DT = "float8e3"               # x dtype (e3m4)
OUT_DT = "bfloat16"

LAST_RESULT = None  # BassKernelResults of the most recent run (for test.py)

_cache = {}


def _build():
    wdt = getattr(mybir.dt, COMPUTE_DT)
    xdt = getattr(mybir.dt, X_DT)
    odt = getattr(mybir.dt, OUT_DT)
    f32 = mybir.dt.float32
    nc = bacc.Bacc("TRN2", target_bir_lowering=False, debug=False,
                   num_devices=N_CORES)
    NQ = KB // 2                   # 16 j-pair chunks
    # xt[q, p, 2048*jj + n] = x[n, 128*(2q+jj) - 8 + p]
    XT = nc.dram_tensor("xt", [NQ, 128, 2 * TPC], xdt,
                        kind="ExternalInput").ap()
    # xtl[p, n] = x[n, 4088 + p] for p < 8 else 0 (zero-padded to 128
    # partitions on-chip)
    XTL = nc.dram_tensor("xtl", [16, TPC], xdt, kind="ExternalInput").ap()
    # wm[p, 128*j + m] = (weight*mask)[128j + m, 128j - 8 + p]
    WM = nc.dram_tensor("wm", [128, KB * 128], wdt,
                        kind="ExternalInput").ap()
    # wh[p, 128*j + m] = (weight*mask)[128j + m, 128j + 120 + p]
    # (zero-padded to K=128 on-chip: DVE memset + DMA into rows 0..15)
    WH = nc.dram_tensor("wh", [16, KB * 128], wdt, kind="ExternalInput").ap()
    BM = nc.dram_tensor("bm", [128, KB], f32, kind="ExternalInput").ap()
    # ou[u, p, jj*TPC + n] = out^T[128*(2u+jj) + p, n]
    OU = nc.dram_tensor("out", [NQ, 128, 2 * TPC], odt,
                        kind="ExternalOutput").ap()

    ident = mybir.ActivationFunctionType.Identity
    add = mybir.AluOpType.add
    WMC = KB * 128 // 4            # wm column chunk (8 j's worth)

    with TileContext(nc) as tc:
        with (
            tc.tile_pool(name="bp", bufs=1) as bp,
            tc.tile_pool(name="xp", bufs=16) as xp,
            tc.tile_pool(name="op", bufs=7) as op,
            tc.tile_pool(name="pp", bufs=8, space="PSUM") as pp,
        ):
            bias_sb = bp.tile([128, KB], f32)
            wm_sb = bp.tile([128, KB * 128], wdt)
            wh_sb = bp.tile([128, KB * 128], wdt)
            xtl_sb = bp.tile([128, TPC], xdt)

            warm_sb = bp.tile([128, 512], wdt)
            # Zero-pad the halo operands to full 128 partitions (keeps
            # every matmul at tile_size (128, 128)). First DVE
            # instructions; they finish (~5 us) before the wh/xtl DMAs
            # can even dispatch, so nothing ever waits on them.
            nc.vector.memset(warm_sb[:], 0)
            nc.vector.memset(wh_sb[:], 0)
            nc.vector.memset(xtl_sb[:], 0)

            xq_sb = {}

            def load_xq(q):
                t = xp.tile([128, 2 * TPC], xdt, tag="xq")
                if q == 0:
                    # split so block j=0's matmuls start as soon as the
                    # first quarter-MB lands
                    nc.sync.dma_start(out=t[:, 0:TPC], in_=XT[0][:, 0:TPC])
                    nc.sync.dma_start(out=t[:, TPC:2 * TPC],
                                      in_=XT[0][:, TPC:2 * TPC])
                else:
                    nc.sync.dma_start(out=t[:], in_=XT[q])
                xq_sb[q] = t

            def x_slice(j, c):
                t = xq_sb[j // 2]
                base = (j % 2) * TPC + 512 * c
                return t[:, base:base + 512]

            def halo_slice(j, c):
                # inputs [128j+120, 128j+136) = partitions 0..15 of
                # block j+1 (xtl for j = KB-1); rows 16..127 are read
                # too but multiplied by the zero-padded weight rows.
                if j == KB - 1:
                    return xtl_sb[:, 512 * c:512 * c + 512]
                t = xq_sb[(j + 1) // 2]
                base = ((j + 1) % 2) * TPC + 512 * c
                return t[:, base:base + 512]

            # All x reads front-load with no compute dependencies (x0
            # first — it gates the first matmul); weight chunks
            # interleave so the first wm columns land before j=0's
            # matmul group needs them.
            load_xq(0)
            nc.sync.dma_start(out=wm_sb[:, 0:WMC], in_=WM[:, 0:WMC])
            nc.sync.dma_start(out=bias_sb[:], in_=BM[:])
            load_xq(1)
            load_xq(2)
            nc.sync.dma_start(out=wh_sb[0:16, :], in_=WH[:])
            load_xq(3)
            load_xq(4)
            nc.sync.dma_start(out=wm_sb[:, WMC:2 * WMC], in_=WM[:, WMC:2 * WMC])
            load_xq(5)
            nc.sync.dma_start(out=wm_sb[:, 2 * WMC:3 * WMC],
                              in_=WM[:, 2 * WMC:3 * WMC])
            load_xq(6)
            nc.sync.dma_start(out=wm_sb[:, 3 * WMC:4 * WMC],
                              in_=WM[:, 3 * WMC:4 * WMC])
            for q in range(7, NQ):
                load_xq(q)
            nc.sync.dma_start(out=xtl_sb[0:16, :], in_=XTL[:])

            # PE warm-up: dummy matmuls on the zeroed scratch tile fill
            # the otherwise-idle first microseconds so the HAM clock
            # gate reaches 2.4 GHz before the first real matmul; a few
            # more are scattered into the early read-fed pairs, whose
            # feed gaps otherwise re-trip the cold state.
            def warm(n, tag):
                for w in range(n):
                    pw = pp.tile([128, 512], f32, tag="ps",
                                 name=f"warm{tag}_{w}")
                    nc.tensor.matmul(pw[:], warm_sb[:, 0:128], warm_sb[:],
                                     start=True, stop=True)

            warm(22, "a")

            oj2 = None
            for j in range(KB):
                q = j // 2
                if j in (4, 8):
                    warm(2, f"b{j}")
                if j % 2 == 0:
                    oj2 = op.tile([128, 2 * TPC], odt, tag="o")
                wmj = wm_sb[:, 128 * j:128 * j + 128]
                whj = wh_sb[:, 128 * j:128 * j + 128]
                ps = [pp.tile([128, 512], f32, tag="ps", name=f"ps{j}_{c}")
                      for c in range(CC)]
                for c in range(CC):
                    nc.tensor.matmul(ps[c][:], wmj, x_slice(j, c),
                                     start=True, stop=False)
                for c in range(CC):
                    nc.tensor.matmul(ps[c][:], whj, halo_slice(j, c),
                                     start=False, stop=True)
                for c in range(CC):
                    ob = (j % 2) * TPC + 512 * c
                    osl = oj2[:, ob:ob + 512]
                    bsl = bias_sb[:, j:j + 1]
                    if (j + c) % 2 == 0:
                        nc.scalar.activation(osl, ps[c][:], ident, bias=bsl)
                    else:
                        nc.vector.tensor_scalar(osl, ps[c][:], bsl, None,
                                                op0=add)
                if j >= KB - 6:
                    # per-j half stores in the compute-paced tail keep
                    # the (otherwise empty) DMA queue supplied
                    jj = j % 2
                    nc.sync.dma_start(out=OU[q][:, jj * TPC:(jj + 1) * TPC],
                                      in_=oj2[:, jj * TPC:(jj + 1) * TPC])
                elif j % 2 == 1:
                    nc.sync.dma_start(out=OU[q], in_=oj2[:])
    nc.finalize()
    return nc


def _get_nc():
    if "nc" not in _cache:
        _cache["nc"] = _build()
    return _cache["nc"]


def kernel(x, weight, bias, mask):
    global LAST_RESULT
    x = np.asarray(x, dtype=np.float32)
    weight = np.asarray(weight, dtype=np.float32)
    bias = np.asarray(bias, dtype=np.float32)
    mask = np.asarray(mask, dtype=np.float32)

    wnp = ml_dtypes.bfloat16
    xnp = ml_dtypes.float8_e3m4
    wm_full = weight * mask                 # [O, I]

    # wm[p, 128j + m] = wm_full[128j + m, 128j - 8 + p]
    wm = np.zeros((128, KB, 128), dtype=np.float32)
    # wh[p, 128j + m] = wm_full[128j + m, 128j + 120 + p]
    wh = np.zeros((16, KB, 128), dtype=np.float32)
    for j in range(KB):
        blk = wm_full[128 * j:128 * j + 128]            # [128, NF]
        lo = 128 * j - 8
        s = max(0, -lo)
        wm[s:, j, :] = blk[:, lo + s:lo + 128].T
        hi = 128 * j + 120
        e = min(16, NF - hi)
        if e > 0:
            wh[:e, j, :] = blk[:, hi:hi + e].T
    wm = np.ascontiguousarray(wm.reshape(128, KB * 128).astype(wnp))
    wh = np.ascontiguousarray(wh.reshape(16, KB * 128).astype(wnp))

    bm = np.ascontiguousarray(bias.reshape(KB, 128).T.astype(np.float32))

    # one f32 -> e3m4 pass over the whole x, then byte-level packing
    xq8 = x.astype(xnp)                                # [N_TOK, NF]
    in_maps = []
    for ci in range(N_CORES):
        xs = xq8[TPC * ci:TPC * (ci + 1)]              # [TPC, NF] e3m4
        xsh = np.zeros((NF + 8, TPC), dtype=xnp)
        xsh[8:] = xs.T                                 # xsh[8 + f, n] = xs[n, f]
        # xt[q, p, 2048*jj + n] = xsh[128*(2q+jj) + p, n]
        xt = (xsh[:NF].reshape(KB // 2, 2, 128, TPC)
              .transpose(0, 2, 1, 3))
        xt = np.ascontiguousarray(xt).reshape(KB // 2, 128, 2 * TPC)
        xtl = np.zeros((16, TPC), dtype=xnp)
        xtl[:8] = xs.T[4088:]
        in_maps.append({
            "xt": xt,
            "xtl": xtl,
            "wm": wm, "wh": wh, "bm": bm,
        })

    nc = _get_nc()
    LAST_RESULT = run_bass_kernel_spmd(nc, in_maps, list(range(N_CORES)))

    out = np.empty((N_TOK, NF), dtype=np.float32)
    for ci in range(N_CORES):
        ou = np.asarray(LAST_RESULT.results[ci]["out"], dtype=np.float32)
        ot = (ou.reshape(KB // 2, 128, 2, TPC).transpose(0, 2, 1, 3)
              .reshape(NF, TPC))
        out[TPC * ci:TPC * (ci + 1)] = ot.T
    return out
